# revision 1
# baseline (speedup 1.0000x reference)
"""Sharded attention kernel for Trainium2 (8 NeuronCores).

Problem: B=2, T=2048, D=1024, H=16 heads (head dim 64), causal self-attention
with separate Q/K/V projections, key-mask additive bias and post-softmax
query-mask, fp32 reference.

Sharding: data-parallel over the 2 batches x tensor-parallel over 4 head
groups (4 heads each) -> 8 fully independent cores, no collectives.

Per-core plan (all matmuls fp32r, 1 cycle/row on the PE):
  - host pre-transposes x (so the d-contraction sits on partitions) and the
    W^T slices; biases/masks/constants are precomputed host-side too.
  - projections produce qT,kT in [head_dim, T] layout and v in natural
    [tk, head_dim] layout with a ones column appended (softmax denominators
    fall out of the PV matmul for free).
  - scores are computed transposed, S_T[tk, tq] = k.q, one 128-row k-strip x
    512-col q-chunk at a time; softmax needs no reductions at all: bounded
    inputs let us skip the max-subtraction, exp runs on the scalar engine
    with the 1/sqrt(64) scale and key-mask bias fused in, and the denominator
    comes from the ones column of v.
  - causality: blocks entirely above the diagonal are skipped, diagonal
    blocks are exp'd only on their live columns and the 128-wide triangle is
    zeroed with one multiplicative mask; the PV matmul accumulates only live
    columns so the dead region is never touched.
  - ctx^T [65, tq] (64 dims + denominator row) is PE-transposed back in 128
    column blocks, normalized by reciprocal(denominator) * query_mask on the
    vector engine, and streamed out.
"""

import os
import sys
import time

import numpy as np

for _p in ("/opt/trn_rl_repo",):
    if os.path.isdir(_p) and _p not in sys.path:
        sys.path.append(_p)

import concourse.bass as bass  # noqa: E402
import concourse.mybir as mybir  # noqa: E402
import concourse.tile as tile  # noqa: E402
from concourse import bacc  # noqa: E402
from concourse.bass_utils import run_bass_kernel_spmd  # noqa: E402

B, T, D, H = 2, 2048, 1024, 16
HD = D // H          # 64 head dim
NCORES = 8
BG = NCORES // B     # 4 head-groups per batch
HG = H // BG         # 4 heads per core
HDG = HG * HD        # 256 projection cols per core
PB = 128             # partition block
NT = T // PB         # 16 k-strips / t-tiles
QC = 512             # q-chunk width
NCH = T // QC        # 4 q-chunks
KC = D // PB         # 8 contraction chunks
SCALE = 1.0 / (HD ** 0.5)

_CACHE: dict = {}
_STAGE = int(os.environ.get("K_STAGE", "3"))  # 1=proj 2=+attn 3=full (debug)
_REPS = int(os.environ.get("K_REPS", "1"))   # repeat body in-NEFF (timing)
_PBUFS = int(os.environ.get("K_PBUFS", "6"))
_XBUFS = int(os.environ.get("K_XBUFS", "16"))
_MMBUFS = int(os.environ.get("K_MMBUFS", "3"))
_ILEAVE = int(os.environ.get("K_ILEAVE", "1"))


def _build(mask_future: bool, qk_bias: bool, v_bias: bool, _stage=None):
    f32 = mybir.dt.float32
    f32r = mybir.dt.float32r
    F = mybir.ActivationFunctionType

    nc = bacc.Bacc("TRN2", target_bir_lowering=False, debug=False,
                   num_devices=NCORES)
    xqT = nc.dram_tensor("xqT", [D, T], f32r, kind="ExternalInput").ap()
    xkT = nc.dram_tensor("xkT", [D, T], f32r, kind="ExternalInput").ap()
    wqT = nc.dram_tensor("wqT", [D, HDG], f32r, kind="ExternalInput").ap()
    wkT = nc.dram_tensor("wkT", [D, HDG], f32r, kind="ExternalInput").ap()
    wvT = nc.dram_tensor("wvT", [D, HDG], f32r, kind="ExternalInput").ap()
    kmb = nc.dram_tensor("kmb", [PB, NT], f32, kind="ExternalInput").ap()
    qm = nc.dram_tensor("qm", [PB, NT], f32, kind="ExternalInput").ap()
    ident = nc.dram_tensor("ident", [PB, PB], f32, kind="ExternalInput").ap()
    ones_g = nc.dram_tensor("ones_g", [PB, HG], f32r, kind="ExternalInput").ap()
    causal = None
    if mask_future:
        causal = nc.dram_tensor("causal", [PB, PB], f32r,
                                kind="ExternalInput").ap()
    bq2 = bk2 = bvb = None
    if qk_bias:
        bq2 = nc.dram_tensor("bq2", [PB, 2], f32, kind="ExternalInput").ap()
        bk2 = nc.dram_tensor("bk2", [PB, 2], f32, kind="ExternalInput").ap()
    if v_bias:
        bvb = nc.dram_tensor("bvb", [PB, HDG], f32, kind="ExternalInput").ap()
    out = nc.dram_tensor("out", [T, HDG], f32, kind="ExternalOutput").ap()

    with tile.TileContext(nc) as tc:
        with (
            tc.tile_pool(name="singles", bufs=1) as singles,
            tc.tile_pool(name="xq", bufs=_XBUFS) as xq_pool,
            tc.tile_pool(name="xk", bufs=_XBUFS) as xk_pool,
            tc.tile_pool(name="qT", bufs=2 * NCH) as qT_pool,
            tc.tile_pool(name="kT", bufs=2 * NCH) as kT_pool,
            tc.tile_pool(name="v", bufs=NT) as v_pool,
            tc.tile_pool(name="pt", bufs=_PBUFS) as p_pool,
            tc.tile_pool(name="ctxs", bufs=int(os.environ.get("K_CTXS", "2"))) as ctxs_pool,
            tc.tile_pool(name="outs", bufs=NT) as outs_pool,
            tc.tile_pool(name="rec", bufs=int(os.environ.get("K_RECS", "4"))) as rec_pool,
            tc.tile_pool(name="pp_a", bufs=_MMBUFS, space="PSUM") as pp_a,
            tc.tile_pool(name="pp_s", bufs=int(os.environ.get("K_SBUFS", "3")), space="PSUM") as pp_s,
            tc.tile_pool(name="pp_ctx", bufs=int(os.environ.get("K_CBUFS", "1")), space="PSUM") as pp_ctx,
            tc.tile_pool(name="pp_t", bufs=int(os.environ.get("K_TBUFS", "1")), space="PSUM") as pp_t,
        ):
            # ---- constants / weights
            w_sb = {}
            for name, src in (("q", wqT), ("k", wkT), ("v", wvT)):
                wt = singles.tile([PB, KC, HDG], f32r, tag=f"w{name}")
                nc.sync.dma_start(
                    out=wt, in_=src.rearrange("(c p) n -> p c n", p=PB))
                w_sb[name] = wt
            km_t = singles.tile([PB, NT], f32, tag="km")
            nc.sync.dma_start(out=km_t, in_=kmb)
            qm_t = singles.tile([PB, NT], f32, tag="qm")
            nc.sync.dma_start(out=qm_t, in_=qm)
            id_t = singles.tile([PB, PB], f32, tag="id")
            nc.sync.dma_start(out=id_t, in_=ident)
            ones_t = singles.tile([PB, HG], f32r, tag="ones")
            nc.sync.dma_start(out=ones_t, in_=ones_g)
            cz_t = None
            if mask_future:
                cz_t = singles.tile([PB, PB], f32r, tag="cz")
                nc.sync.dma_start(out=cz_t, in_=causal)
            bq_t = bk_t = bv_t = None
            if qk_bias:
                bq_t = singles.tile([PB, 2], f32, tag="bq")
                nc.sync.dma_start(out=bq_t, in_=bq2)
                bk_t = singles.tile([PB, 2], f32, tag="bk")
                nc.sync.dma_start(out=bk_t, in_=bk2)
            if v_bias:
                bv_t = singles.tile([PB, HDG], f32, tag="bv")
                nc.sync.dma_start(out=bv_t, in_=bvb)

            for rep in range(_REPS):
                # ---- projections
                # per-(head-tile, chunk) tiles keep dependencies
                # chunk-granular so attention overlaps projections
                qT_sb = {(ht, ch): qT_pool.tile([PB, QC], f32r, tag="qT",
                                                name=f"qT{rep}_{ht}_{ch}")
                         for ht in range(2) for ch in range(NCH)}
                kT_sb = {(ht, ch): kT_pool.tile([PB, QC], f32r, tag="kT",
                                                name=f"kT{rep}_{ht}_{ch}")
                         for ht in range(2) for ch in range(NCH)}
                v_sb = [v_pool.tile([PB, HG * (HD + 1)], f32r, tag="v",
                                     name=f"v{rep}_{i}") for i in range(NT)]

                out_sb = [outs_pool.tile([PB, HDG], f32, tag="o",
                                           name=f"os{rep}_{i}") for i in range(NT)]

                def attn_chunk(j, heads=tuple(range(HG))):
                    for h in heads:
                        ht, off = h // 2, (h % 2) * HD
                        qch = qT_sb[(ht, j)][off:off + HD, :]
                        cps = pp_ctx.tile([HD + 1, QC], f32, tag="ctx",
                                          name=f"cps{rep}_{j}_{h}")
                        if mask_future:
                            diag0 = j * (QC // PB)
                            order = list(range(diag0, diag0 + QC // PB)) + \
                                list(range(0, diag0))
                        else:
                            diag0 = NT
                            order = list(range(NT))
                        # software-pipelined: emit S(i+1) before PV(i) so
                        # the PE streams scores while ACT exponentiates the
                        # previous strip instead of stalling on it.
                        pend = None
                        for si, i in enumerate(order):
                            c0 = 0
                            if mask_future and i >= diag0:
                                c0 = (i - diag0) * PB
                            sps = pp_s.tile([PB, QC], f32, tag="s",
                                            name=f"sps{rep}_{j}_{h}_{i}")
                            nc.tensor.matmul(
                                sps[:, c0:QC],
                                kT_sb[(ht, i // (QC // PB))][
                                    off:off + HD,
                                    (i % (QC // PB)) * PB:
                                    (i % (QC // PB) + 1) * PB],
                                qch[:, c0:QC], start=True, stop=True)
                            pt = p_pool.tile([PB, QC], f32r, tag="p",
                                             name=f"pt{rep}_{j}_{h}_{i}")
                            nc.scalar.activation(
                                out=pt[:, c0:QC], in_=sps[:, c0:QC],
                                func=F.Exp,
                                bias=km_t[:, i:i + 1], scale=SCALE)
                            if mask_future and i >= diag0:
                                nc.vector.tensor_mul(
                                    pt[:, c0:c0 + PB], pt[:, c0:c0 + PB],
                                    cz_t)
                            if pend is not None:
                                psi, pi, pc0, ppt = pend
                                nc.tensor.matmul(
                                    cps[:, pc0:QC],
                                    v_sb[pi][:, h * (HD + 1):
                                             (h + 1) * (HD + 1)],
                                    ppt[:, pc0:QC],
                                    start=(psi == 0), stop=False)
                            pend = (si, i, c0, pt)
                        psi, pi, pc0, ppt = pend
                        nc.tensor.matmul(
                            cps[:, pc0:QC],
                            v_sb[pi][:, h * (HD + 1):(h + 1) * (HD + 1)],
                            ppt[:, pc0:QC],
                            start=(psi == 0), stop=True)
                        if _STAGE < 3:
                            continue
                        csb = ctxs_pool.tile([HD + 1, QC], f32, tag="c",
                                             name=f"csb{rep}_{j}_{h}")
                        nc.vector.tensor_copy(csb, cps)
                        for r in range(QC // PB):
                            jt = j * (QC // PB) + r
                            tp = pp_t.tile([PB, HD + 1], f32, tag="t",
                                           name=f"tp{rep}_{j}_{h}_{r}")
                            nc.tensor.matmul(
                                tp, csb[:, r * PB:(r + 1) * PB],
                                id_t[0:HD + 1, 0:HD + 1], is_transpose=True)
                            rc = rec_pool.tile([PB, 1], f32, tag="r",
                                               name=f"rc{rep}_{j}_{h}_{r}")
                            nc.vector.reciprocal(rc, tp[:, HD:HD + 1])
                            nc.vector.tensor_mul(rc, rc, qm_t[:, jt:jt + 1])
                            nc.vector.tensor_scalar_mul(
                                out_sb[jt][:, h * HD:(h + 1) * HD],
                                tp[:, 0:HD], rc)

                def store_chunk(j):
                    if _STAGE < 3:
                        return
                    for jt in range(j * (QC // PB), (j + 1) * (QC // PB)):
                        nc.sync.dma_start(
                            out=out[jt * PB:(jt + 1) * PB, :], in_=out_sb[jt])

                for ch in range(NCH):
                    xq_ts, xk_ts = [], []
                    for dc in range(KC):
                        # one 512 KB DMA per (dc, ch) pair loads q and k
                        # halves into one tile: fewer, larger transfers
                        t1 = xq_pool.tile([PB, QC], f32r, tag="xq")
                        nc.sync.dma_start(
                            out=t1,
                            in_=xqT[dc * PB:(dc + 1) * PB,
                                    ch * QC:(ch + 1) * QC])
                        xq_ts.append(t1)
                        t2 = xk_pool.tile([PB, QC], f32r, tag="xk")
                        nc.sync.dma_start(
                            out=t2,
                            in_=xkT[dc * PB:(dc + 1) * PB,
                                    ch * QC:(ch + 1) * QC])
                        xk_ts.append(t2)
                    def proj_qk(ht):
                        # q/k projection for one head-tile:
                        # out = W^T.T @ x^T -> [head_dim, tq]
                        for wname, x_ts, dst, bias_t in (
                                ("q", xq_ts, qT_sb, bq_t),
                                ("k", xk_ts, kT_sb, bk_t)):
                            ps = pp_a.tile([PB, QC], f32, tag="mm",
                                           name=f"pp{rep}_{ch}_{wname}_{ht}")
                            for dc in range(KC):
                                nc.tensor.matmul(
                                    ps,
                                    w_sb[wname][:, dc, ht * PB:(ht + 1) * PB],
                                    x_ts[dc],
                                    start=(dc == 0), stop=(dc == KC - 1))
                            dslice = dst[(ht, ch)][:, :]
                            if bias_t is not None:
                                nc.vector.tensor_scalar_add(
                                    dslice, ps, bias_t[:, ht:ht + 1])
                            else:
                                nc.vector.tensor_copy(dslice, ps)

                    def proj_v():
                        # v projection: natural layout, x^T chunk stationary
                        for r in range(QC // PB):
                            tk = ch * (QC // PB) + r
                            ps = pp_a.tile([PB, QC], f32, tag="mm",
                                           name=f"pv{rep}_{ch}_{r}")
                            for dc in range(KC):
                                nc.tensor.matmul(
                                    ps[:, 0:HDG],
                                    xk_ts[dc][:, r * PB:(r + 1) * PB],
                                    w_sb["v"][:, dc, :],
                                    start=(dc == 0), stop=(dc == KC - 1))
                            v3 = v_sb[tk].rearrange("p (g c) -> p g c",
                                                    c=HD + 1)
                            ps3 = ps[:, 0:HDG].rearrange("p (g c) -> p g c",
                                                         c=HD)
                            if bv_t is not None:
                                nc.vector.tensor_add(
                                    v3[:, :, 0:HD], ps3,
                                    bv_t.rearrange("p (g c) -> p g c", c=HD))
                            else:
                                nc.vector.tensor_copy(v3[:, :, 0:HD], ps3)
                            nc.vector.tensor_copy(
                                v3[:, :, HD:HD + 1],
                                ones_t.rearrange("p (g o) -> p g o", o=1))

                    if mask_future and _ILEAVE and _STAGE >= 2:
                        # heads 0-1 only need the ht=0 projections + v, so
                        # emit them before the ht=1 projections: the scalar
                        # engine (attention's critical engine) starts each
                        # chunk's exps ~25% earlier
                        proj_qk(0)
                        proj_v()
                        attn_chunk(ch, heads=(0, 1))
                        proj_qk(1)
                        attn_chunk(ch, heads=(2, 3))
                        store_chunk(ch)
                    else:
                        proj_qk(0)
                        proj_qk(1)
                        proj_v()

                if (not mask_future or not _ILEAVE) and _STAGE >= 2:
                    for j in range(NCH):
                        attn_chunk(j)
                        store_chunk(j)

    nc.compile()
    return nc


def _get_nc(mask_future: bool, qk_bias: bool, v_bias: bool):
    key = (mask_future, qk_bias, v_bias, _STAGE, _REPS, _PBUFS, _XBUFS, _MMBUFS, _ILEAVE)
    if key not in _CACHE:
        _CACHE[key] = _build(*key[:3])
    return _CACHE[key]


def _in_maps(query_states, key_states, query_mask, key_mask,
             Wq, bq, Wk, bk, Wv, bv, mask_future, qk_bias, v_bias):
    f4 = np.float32
    ident = np.eye(PB, dtype=f4)
    ones_g = np.ones((PB, HG), dtype=f4)
    causal = np.triu(np.ones((PB, PB), dtype=f4))
    in_maps = []
    for c in range(NCORES):
        b, g = c // BG, c % BG
        s = slice(g * HDG, (g + 1) * HDG)
        m = {
            "xqT": np.ascontiguousarray(query_states[b].T, dtype=f4),
            "xkT": np.ascontiguousarray(key_states[b].T, dtype=f4),
            "wqT": np.ascontiguousarray(Wq[s, :].T, dtype=f4),
            "wkT": np.ascontiguousarray(Wk[s, :].T, dtype=f4),
            "wvT": np.ascontiguousarray(Wv[s, :].T, dtype=f4),
            "kmb": np.ascontiguousarray(
                ((np.asarray(key_mask[b], f4) - 1.0) * 10000.0)
                .reshape(NT, PB).T),
            "qm": np.ascontiguousarray(
                np.asarray(query_mask[b], f4).reshape(NT, PB).T),
            "ident": ident,
            "ones_g": ones_g,
        }
        if mask_future:
            m["causal"] = causal
        if qk_bias:
            m["bq2"] = np.ascontiguousarray(
                np.asarray(bq[s], f4).reshape(2, PB).T)
            m["bk2"] = np.ascontiguousarray(
                np.asarray(bk[s], f4).reshape(2, PB).T)
        if v_bias:
            m["bvb"] = np.ascontiguousarray(
                np.broadcast_to(np.asarray(bv[s], f4), (PB, HDG)))
        in_maps.append(m)
    return in_maps


def kernel(query_states, key_states, query_mask, key_mask,
           Wq, bq, Wk, bk, Wv, bv, mask_future):
    query_states = np.asarray(query_states, np.float32)
    key_states = np.asarray(key_states, np.float32)
    mask_future = bool(int(np.asarray(mask_future)))
    qk_bias = bool(np.any(np.asarray(bq)) or np.any(np.asarray(bk)))
    v_bias = bool(np.any(np.asarray(bv)))

    nc = _get_nc(mask_future, qk_bias, v_bias)
    in_maps = _in_maps(query_states, key_states, query_mask, key_mask,
                       Wq, bq, Wk, bk, Wv, bv, mask_future, qk_bias, v_bias)
    res = run_bass_kernel_spmd(nc, in_maps, core_ids=list(range(NCORES)))
    full = np.empty((B, T, D), np.float32)
    for c in range(NCORES):
        b, g = c // BG, c % BG
        full[b][:, g * HDG:(g + 1) * HDG] = res.results[c]["out"]
    return full


# ---------------------------------------------------------------------------
# helpers for test.py (not used by the grader)

_RUNNER_CACHE: dict = {}


def timed_run(inputs, iters=10):
    """Run the kernel repeatedly through one jitted PJRT executable and
    return (first_results_full_output, list of per-iter wall seconds)."""
    import jax
    from jax.sharding import Mesh, PartitionSpec
    from jax.experimental.shard_map import shard_map
    from concourse import bass2jax

    mask_future = bool(int(np.asarray(inputs["mask_future"])))
    qk_bias = bool(np.any(np.asarray(inputs["bq"])) or
                   np.any(np.asarray(inputs["bk"])))
    v_bias = bool(np.any(np.asarray(inputs["bv"])))
    nc = _get_nc(mask_future, qk_bias, v_bias)
    if id(nc) in _RUNNER_CACHE:
        sharded, dev_args, out_names, in_names = _RUNNER_CACHE[id(nc)]
        return _run_timed(sharded, dev_args, out_names, iters)
    in_maps = _in_maps(
        np.asarray(inputs["query_states"], np.float32),
        np.asarray(inputs["key_states"], np.float32),
        inputs["query_mask"], inputs["key_mask"],
        inputs["Wq"], inputs["bq"], inputs["Wk"], inputs["bk"],
        inputs["Wv"], inputs["bv"], mask_future, qk_bias, v_bias)

    bass2jax.install_neuronx_cc_hook()
    partition_name = (nc.partition_id_tensor.name
                      if nc.partition_id_tensor else None)
    in_names, out_names, out_avals, zero_outs = [], [], [], []
    for alloc in nc.m.functions[0].allocations:
        if not isinstance(alloc, mybir.MemoryLocationSet):
            continue
        name = alloc.memorylocations[0].name
        if alloc.kind == "ExternalInput":
            if name != partition_name:
                in_names.append(name)
        elif alloc.kind == "ExternalOutput":
            out_names.append(name)
            shape = tuple(alloc.tensor_shape)
            dtype = mybir.dt.np(alloc.dtype)
            out_avals.append(jax.core.ShapedArray(shape, dtype))
            zero_outs.append(np.zeros(shape, dtype))
    n_params = len(in_names)
    all_names = in_names + out_names
    if partition_name is not None:
        all_names.append(partition_name)

    def _body(*args):
        operands = list(args)
        if partition_name is not None:
            operands.append(bass2jax.partition_id_tensor())
        outs = bass2jax._bass_exec_p.bind(
            *operands, out_avals=tuple(out_avals), in_names=tuple(all_names),
            out_names=tuple(out_names), lowering_input_output_aliases=(),
            sim_require_finite=True, sim_require_nnan=True, nc=nc)
        return tuple(outs)

    devices = jax.devices()[:NCORES]
    mesh = Mesh(np.asarray(devices), ("core",))
    n_outs = len(out_names)
    sharded = jax.jit(
        shard_map(_body, mesh=mesh,
                  in_specs=(PartitionSpec("core"),) * (n_params + n_outs),
                  out_specs=(PartitionSpec("core"),) * n_outs,
                  check_rep=False),
        keep_unused=True)
    concat_in = [np.concatenate([np.asarray(in_maps[c][n]) for c in
                                 range(NCORES)], axis=0)
                 for n in in_names]
    concat_zeros = [np.zeros((NCORES * z.shape[0], *z.shape[1:]), z.dtype)
                    for z in zero_outs]
    dev_args = [jax.device_put(a) for a in concat_in + concat_zeros]
    _RUNNER_CACHE[id(nc)] = (sharded, dev_args, out_names, in_names)
    return _run_timed(sharded, dev_args, out_names, iters)


def _run_timed(sharded, dev_args, out_names, iters):
    import jax
    outs = sharded(*dev_args)
    jax.block_until_ready(outs)
    times = []
    for _ in range(iters):
        t0 = time.perf_counter()
        outs = sharded(*dev_args)
        jax.block_until_ready(outs)
        times.append(time.perf_counter() - t0)
    full = np.empty((B, T, D), np.float32)
    arr = np.asarray(outs[out_names.index("out")]).reshape(NCORES, T, HDG)
    for c in range(NCORES):
        b, g = c // BG, c % BG
        full[b][:, g * HDG:(g + 1) * HDG] = arr[c]
    return full, times


def modeled_time_ns():
    """Cost-model (TimelineSim) estimate for the current cached module."""
    from concourse.timeline_sim import TimelineSim
    nc = next(iter(_CACHE.values()))
    return TimelineSim(nc, no_exec=True).simulate()



# revision 9
# speedup vs baseline: 1.3064x; 1.3064x over previous
"""Sharded attention kernel for Trainium2 (8 NeuronCores), v5.

Problem: B=2, T=2048, D=1024, H=16 heads (head dim 64), causal self-attention
with separate Q/K/V projections, key-mask additive bias and post-softmax
query-mask, fp32 reference; tolerance max-rel 2e-2 vs global max.

Sharding: data-parallel over the 2 batches x tensor-parallel over 4 head
groups (4 heads each) -> 8 fully independent cores, no collectives.

Per-core plan (all big matmuls fp8e4m3 DoubleRow = 0.5 cycles/row with 256-
deep contraction; accuracy recovered by an fp16 path for rows < 512 whose
softmax support is small):
  - Q/K projections emit q^T/k^T directly in the DoubleRow d-split layout
    [128p = 4 heads x 32 d_low, 2 slots, t] via host-packed weight column
    order; V projection emits natural [t_k, dims] strips. fp16 plain-layout
    projections cover chunk-0 q/k and v strips 0-3 (when causal).
  - scores S^T[k, q] per 128-row k-strip: fp8-DR (256 cyc per 512-wide
    strip), fp16 non-DR for chunk-0 diagonal. The causal triangle of a
    diagonal block is handled by a -10000 bias matmul into PSUM (ACT-exp
    path) or a {A,0} mask operand inside the DVE Schraudolph.
  - softmax: no max-subtraction (scores bounded); exp work is split between
    the ACT engine (true exp, fp8 output) and the DVE (Schraudolph int8
    bit-trick: round(1.4427*s + 55.65) written as int8 == fp8e4m3 bits of
    ~exp(s/8); hw rounds-to-nearest and saturates; masked cells -> 0).
    Key-mask bias exp(b_k) is folded multiplicatively into the V rows
    (including the denominator ones-column), so exp needs no bias operand
    and strip-pairs fuse into single [128, 1024] instructions.
  - PV runs in natural layout: probs (stationary, [128k, 2, 128t] slices of
    pair tiles) x V (moving, [128k, 2, 68]) -> ctx[t, dims] accumulated over
    strips; 34 cycles per pair per 128-row t-block. The ones-column gives
    the denominator in ctx column 64; normalization is a per-partition
    reciprocal + one stride-0-broadcast multiply per (chunk, t-block).
"""

import os
import sys

import numpy as np

for _p in ("/opt/trn_rl_repo",):
    if os.path.isdir(_p) and _p not in sys.path:
        sys.path.append(_p)

import ml_dtypes  # noqa: E402

import concourse.mybir as mybir  # noqa: E402
import concourse.tile as tile  # noqa: E402
from concourse import bacc  # noqa: E402
from concourse.bass_utils import run_bass_kernel_spmd  # noqa: E402

B, T, D, H = 2, 2048, 1024, 16
HD = D // H          # 64 head dim
NCORES = 8
BG = NCORES // B     # 4 head-groups per batch
HG = H // BG         # 4 heads per core
HDG = HG * HD        # 256 projection cols per core
PB = 128             # partition block
NT = T // PB         # 16 k-strips
QC = 512             # q-chunk width
NCH = T // QC        # 4 q-chunks
KC = D // PB         # 8 contraction chunks
VW = 68              # padded v width: 64 dims + ones col + 3 pad (4B align)
SCALE = 0.125

# Schraudolph fp8e4m3 bit-trick constants (hw float->int8 rounds to nearest)
ASCH = 8.0 * 1.4426950408889634 * SCALE     # = 1.4427 on raw scores
BSCH = 56.0 - 0.35                           # exponent-bias offset, tuned

f32 = mybir.dt.float32
fp8 = mybir.dt.float8e4
fp16 = mybir.dt.float16
bf16 = mybir.dt.bfloat16
i8 = mybir.dt.int8
F = mybir.ActivationFunctionType
DR = mybir.MatmulPerfMode.DoubleRow
OP = mybir.AluOpType

_CACHE: dict = {}
_REPS = int(os.environ.get("K_REPS", "1"))
# percentage of off-diagonal exp pairs handled by ACT (rest on DVE)
_EXPA = int(os.environ.get("K_EXPA", "90"))
_PT8B = int(os.environ.get("K_PT8B", "60"))
_PTBB = int(os.environ.get("K_PTBB", "17"))
_SBUFS = int(os.environ.get("K_SB", "2"))
_CTXB = int(os.environ.get("K_CTX", "2"))
_OUTB = int(os.environ.get("K_OUTB", "4"))
_PRJB = int(os.environ.get("K_PRJ", "2"))
_X8B = int(os.environ.get("K_X8B", "4"))


def _build(mask_future: bool, qk_bias: bool, v_bias: bool, km_ones: bool,
           qm_ones: bool):
    nc = bacc.Bacc("TRN2", target_bir_lowering=False, debug=False,
                   num_devices=NCORES)
    MF = mask_future
    NQ8 = NCH - 1 if MF else NCH       # q8 chunks (chunk 0 via fp16 if MF)

    def din(name, shape, dt):
        return nc.dram_tensor(name, shape, dt, kind="ExternalInput").ap()

    xq8 = din("xq8", [PB, NQ8, KC, QC], fp8)
    xk8 = din("xk8", [PB, NCH, KC, QC], fp8)
    wq8 = din("wq8", [PB, KC, 2, PB], fp8)
    wk8 = din("wk8", [PB, KC, 2, PB], fp8)
    wv8 = din("wv8", [PB, KC, HDG], fp8)
    xq16 = xk16 = wq16 = wk16 = wv16 = c16 = id16 = cza = None
    if MF:
        xq16 = din("xq16", [PB, KC, QC], fp16)         # chunk 0
        xk16 = din("xk16", [PB, KC, QC], fp16)
        wq16 = din("wq16", [PB, KC, 2, PB], fp16)
        wk16 = din("wk16", [PB, KC, 2, PB], fp16)
        wv16 = din("wv16", [PB, KC, HDG], fp16)
        c16 = din("c16", [PB, PB], fp16)               # upper-tri -10000
        id16 = din("id16", [PB, PB], fp16)
        cza = din("cza", [PB, NCH, QC], f32)           # {ASCH, 0} masks
    ekm = qmm = None
    if not km_ones:
        ekm = din("ekm", [PB, NT], f32)
    if not qm_ones:
        qmm = din("qmm", [PB, NT], f32)
    bq8 = bk8 = bq16 = bk16 = bvr = ones_r = None
    if qk_bias or v_bias:
        ones_r = din("ones_r", [1, QC], fp16)
    if qk_bias:
        bq8 = din("bq8", [1, 2, PB], fp16)   # d-split order biases
        bk8 = din("bk8", [1, 2, PB], fp16)
        if MF:
            bq16 = din("bq16", [1, 2, PB], fp16)  # plain order
            bk16 = din("bk16", [1, 2, PB], fp16)
    if v_bias:
        bvr = din("bvr", [1, HDG], fp16)

    out = nc.dram_tensor("out", [T, HDG], f32, kind="ExternalOutput").ap()

    with tile.TileContext(nc) as tc:
        with (
            tc.tile_pool(name="singles", bufs=1) as S,
            tc.tile_pool(name="x8", bufs=_X8B) as x8_pool,
            tc.tile_pool(name="qT8", bufs=(2 if MF else NCH)) as qT8_pool,
            tc.tile_pool(name="pt8", bufs=_PT8B) as pt8_pool,
            tc.tile_pool(name="ptb", bufs=_PTBB) as ptb_pool,
            tc.tile_pool(name="outs", bufs=_OUTB) as out_pool,
            tc.tile_pool(name="rc", bufs=4) as rc_pool,
            tc.tile_pool(name="pp_prj", bufs=_PRJB, space="PSUM") as pp_prj,
            tc.tile_pool(name="pp_s", bufs=_SBUFS, space="PSUM") as pp_s,
            tc.tile_pool(name="pp_ctx", bufs=_CTXB, space="PSUM") as pp_ctx,
        ):
            # ---------------- constants / weights / persistent tiles
            # fp16 chunk-0 inputs first: they head the critical path
            w = {}
            xq16_t = xk16_t = c16_t = id16_t = cza_t = None
            if MF:
                w["q16"] = S.tile([PB, KC, 2, PB], fp16, tag="wq16", name="wq16t")
                nc.sync.dma_start(out=w["q16"], in_=wq16)
                xq16_t = S.tile([PB, KC, QC], fp16, tag="xq16")
                nc.sync.dma_start(out=xq16_t, in_=xq16)
                w["k16"] = S.tile([PB, KC, 2, PB], fp16, tag="wk16", name="wk16t")
                nc.sync.dma_start(out=w["k16"], in_=wk16)
                xk16_t = S.tile([PB, KC, QC], fp16, tag="xk16")
                nc.sync.dma_start(out=xk16_t, in_=xk16)
                w["v16"] = S.tile([PB, KC, HDG], fp16, tag="wv16", name="wv16t")
                nc.sync.dma_start(out=w["v16"], in_=wv16)
                c16_t = S.tile([PB, PB], fp16, tag="c16")
                nc.sync.dma_start(out=c16_t, in_=c16)
                id16_t = S.tile([PB, PB], fp16, tag="id16")
                nc.sync.dma_start(out=id16_t, in_=id16)
                cza_t = S.tile([PB, NCH, QC], f32, tag="cza")
                nc.sync.dma_start(out=cza_t, in_=cza)
            for name, src, shp, dt in (
                    ("q8", wq8, [PB, KC, 2, PB], fp8),
                    ("k8", wk8, [PB, KC, 2, PB], fp8),
                    ("v8", wv8, [PB, KC, HDG], fp8)):
                t = S.tile(shp, dt, tag=f"w{name}")
                nc.sync.dma_start(out=t, in_=src)
                w[name] = t
            ekm_t = qmm_t = None
            if not km_ones:
                ekm_t = S.tile([PB, NT], f32, tag="ekm")
                nc.sync.dma_start(out=ekm_t, in_=ekm)
            if not qm_ones:
                qmm_t = S.tile([PB, NT], f32, tag="qmm")
                nc.sync.dma_start(out=qmm_t, in_=qmm)
            ones_t = None
            if ones_r is not None:
                ones_t = S.tile([1, QC], fp16, tag="ones")
                nc.sync.dma_start(out=ones_t, in_=ones_r)
            b_t = {}
            for nm, src in (("bq8", bq8), ("bk8", bk8), ("bq16", bq16),
                            ("bk16", bk16)):
                if src is not None:
                    b_t[nm] = S.tile([1, 2, PB], fp16, tag=nm)
                    nc.sync.dma_start(out=b_t[nm], in_=src)
            if bvr is not None:
                b_t["bvr"] = S.tile([1, HDG], fp16, tag="bvr")
                nc.sync.dma_start(out=b_t["bvr"], in_=bvr)

            for rep in range(_REPS):
                kT8 = [S.tile([PB, 2, QC], fp8, tag=f"kT8_{c}",
                              name=f"kT8_{rep}_{c}") for c in range(NCH)]
                kT16 = qT16 = v16 = None
                if MF:
                    kT16 = [S.tile([PB, QC], fp16, tag=f"kT16_{i}",
                                   name=f"kT16_{rep}_{i}") for i in range(2)]
                    qT16 = [S.tile([PB, QC], fp16, tag=f"qT16_{i}",
                                   name=f"qT16_{rep}_{i}") for i in range(2)]
                    v16 = [S.tile([PB, HG, VW], bf16, tag=f"v16_{r}",
                                  name=f"v16_{rep}_{r}") for r in range(4)]
                v8p = [S.tile([PB, 2, HG, VW], fp8, tag=f"v8p_{t}",
                              name=f"v8p_{rep}_{t}") for t in range(NT // 2)]
                qT8 = {}  # ch -> tile

                def scalar_km(i):
                    if km_ones:
                        return None
                    return ekm_t[:, i:i + 1]

                # ---------------- projections
                def _store_v8(i, ps):
                    t, s = i // 2, i % 2
                    km = scalar_km(i)
                    ps3 = ps[:, 0:HDG].rearrange("p (g c) -> p g c", c=HD)
                    if km is None:
                        nc.vector.tensor_copy(v8p[t][:, s, :, 0:HD], ps3)
                        nc.gpsimd.memset(v8p[t][:, s, :, HD:HD + 1], 1.0)
                    else:
                        nc.vector.tensor_scalar_mul(
                            v8p[t][:, s, :, 0:HD], ps3, km)
                        nc.vector.tensor_copy(
                            v8p[t][:, s, :, HD:HD + 1],
                            km.rearrange("p () -> p () ()")
                            .broadcast_to([PB, HG, 1]))

                def proj16_qk():
                    for ht, (wname, x_t, dst, bias) in (
                            (0, ("q16", xq16_t, qT16, "bq16")),
                            (0, ("k16", xk16_t, kT16, "bk16")),
                            (1, ("q16", xq16_t, qT16, "bq16")),
                            (1, ("k16", xk16_t, kT16, "bk16"))):
                        if True:
                            ps = pp_prj.tile([PB, QC], f32, tag="prj",
                                             name=f"p16{rep}_{wname}_{ht}")
                            for dc in range(KC):
                                nc.tensor.matmul(
                                    ps, w[wname][:, dc, ht, :],
                                    x_t[:, dc, :],
                                    start=(dc == 0),
                                    stop=(dc == KC - 1 and not qk_bias))
                            if qk_bias:
                                nc.tensor.matmul(
                                    ps, b_t[bias][:, ht, :], ones_t,
                                    start=False, stop=True)
                            nc.vector.tensor_copy(dst[ht], ps)

                def proj16_v():
                    for r in range(4):
                        ps = pp_prj.tile([PB, QC], f32, tag="prj",
                                         name=f"pv16{rep}_{r}")
                        for dc in range(KC):
                            nc.tensor.matmul(
                                ps[:, 0:HDG],
                                xk16_t[:, dc, r * PB:(r + 1) * PB],
                                w["v16"][:, dc, :],
                                start=(dc == 0),
                                stop=(dc == KC - 1 and not v_bias))
                        if v_bias:
                            nc.tensor.matmul(
                                ps[:, 0:HDG], ones_t[:, 0:PB], b_t["bvr"],
                                start=False, stop=True)
                        km = scalar_km(r)
                        ps3 = ps[:, 0:HDG].rearrange("p (g c) -> p g c", c=HD)
                        if km is None:
                            nc.vector.tensor_copy(v16[r][:, :, 0:HD], ps3)
                            nc.gpsimd.memset(v16[r][:, :, HD:HD + 1], 1.0)
                        else:
                            nc.vector.tensor_scalar_mul(
                                v16[r][:, :, 0:HD], ps3, km)
                            nc.vector.tensor_copy(
                                v16[r][:, :, HD:HD + 1],
                                km.rearrange("p () -> p () ()")
                                .broadcast_to([PB, HG, 1]))
                        _store_v8(r, ps)

                def proj8_qk(ch, do_q=True):
                    """DR projections for chunk ch; returns k x-tile for the
                    v projection. q skipped for chunk 0 when causal."""
                    jobs = []
                    if do_q:
                        qt = qT8_pool.tile([PB, 2, QC], fp8, tag="qT8",
                                           name=f"qT8_{rep}_{ch}")
                        qT8[ch] = qt
                        jobs.append(("q8", xq8[:, ch - 1 if MF else ch],
                                     qt, "bq8"))
                    jobs.append(("k8", xk8[:, ch], kT8[ch], "bk8"))
                    xk_t = None
                    for wname, src, dst, bias in jobs:
                        x_t = x8_pool.tile([PB, KC, QC], fp8, tag="x8",
                                           name=f"x8_{rep}_{ch}_{wname}")
                        nc.sync.dma_start(out=x_t, in_=src)
                        if wname == "k8":
                            xk_t = x_t
                        for s in range(2):
                            ps = pp_prj.tile([PB, QC], f32, tag="prj",
                                             name=f"p8{rep}_{ch}_{wname}_{s}")
                            for d2 in range(KC // 2):
                                nc.tensor.matmul(
                                    ps,
                                    w[wname][:, 2 * d2:2 * d2 + 2, s, :],
                                    x_t[:, 2 * d2:2 * d2 + 2, :],
                                    start=(d2 == 0),
                                    stop=(d2 == KC // 2 - 1 and not qk_bias),
                                    perf_mode=DR)
                            if qk_bias:
                                nc.tensor.matmul(
                                    ps, b_t[bias][:, s, :], ones_t,
                                    start=False, stop=True)
                            nc.vector.tensor_copy(dst[:, s, :], ps)
                    return xk_t

                def proj8_v(ch, xk_t):
                    for r in range(4):
                        i = 4 * ch + r
                        if MF and ch == 0:
                            continue  # strips 0-3 via fp16 path
                        ps = pp_prj.tile([PB, QC], f32, tag="prj",
                                         name=f"pv{rep}_{i}")
                        for d2 in range(KC // 2):
                            nc.tensor.matmul(
                                ps[:, 0:HDG],
                                xk_t[:, 2 * d2:2 * d2 + 2,
                                     r * PB:(r + 1) * PB],
                                w["v8"][:, 2 * d2:2 * d2 + 2, :],
                                start=(d2 == 0),
                                stop=(d2 == KC // 2 - 1 and not v_bias),
                                perf_mode=DR)
                        if v_bias:
                            nc.tensor.matmul(
                                ps[:, 0:HDG], ones_t[:, 0:PB], b_t["bvr"],
                                start=False, stop=True)
                        _store_v8(i, ps)

                # ---------------- attention
                _acc = [0.0]

                def pick_engine():
                    _acc[0] += _EXPA / 100.0
                    if _acc[0] >= 1.0:
                        _acc[0] -= 1.0
                        return "act"
                    return "dve"

                def attn_scores(ch, bg=None):
                    nfull = 2 * ch if MF else NT // 2
                    ndiag = 2 if MF else 0
                    pts = {}
                    ptbs = {}
                    nemit = [0]

                    def tick():
                        nemit[0] += 1
                        if bg and nemit[0] % 2 == 0 and bg:
                            bg.pop(0)()
                    for h in range(HG):
                        if MF and ch == 0:
                            # fp16 scores + ACT exp -> bf16, per strip
                            for tp in range(2):
                                ps = pp_s.tile(
                                    [PB, 2, QC], f32, tag="s",
                                    name=f"s{rep}_0_{h}_{tp}")
                                for rr in range(2):
                                    r = 2 * tp + rr
                                    c0 = r * PB
                                    h2, hb = h // 2, (h % 2) * HD
                                    nc.tensor.matmul(
                                        ps[:, rr, c0:QC],
                                        kT16[h2][hb:hb + HD,
                                                 r * PB:(r + 1) * PB],
                                        qT16[h2][hb:hb + HD, c0:QC],
                                        start=True, stop=False)
                                    nc.tensor.matmul(
                                        ps[:, rr, c0:c0 + PB],
                                        id16_t, c16_t,
                                        start=False, stop=True)
                                    pb = ptb_pool.tile(
                                        [PB, QC], bf16, tag="pb",
                                        name=f"pb{rep}_{h}_{r}")
                                    nc.scalar.activation(
                                        out=pb[:, c0:QC],
                                        in_=ps[:, rr, c0:QC],
                                        func=F.Exp, scale=SCALE)
                                    ptbs[(h, r)] = pb
                                tick()
                            continue
                        for tp in range(nfull + ndiag):
                            diag = MF and tp >= nfull
                            ps = pp_s.tile([PB, 2, QC], f32, tag="s",
                                           name=f"s{rep}_{ch}_{h}_{tp}")
                            pt = pt8_pool.tile([PB, 2, QC], fp8, tag="pt",
                                               name=f"pt{rep}_{ch}_{h}_{tp}")
                            pts[(h, tp)] = pt
                            for rr in range(2):
                                i = 2 * tp + rr
                                c0 = (i - 4 * ch) * PB if diag else 0
                                ck, rb = i // 4, i % 4
                                nc.tensor.matmul(
                                    ps[:, rr, c0:QC],
                                    kT8[ck][32 * h:32 * h + 32, :,
                                            rb * PB:(rb + 1) * PB],
                                    qT8[ch][32 * h:32 * h + 32, :, c0:QC],
                                    start=True, stop=True, perf_mode=DR,
                                    tile_position=(32 * h, 0))
                            if not diag:
                                if pick_engine() == "act":
                                    nc.scalar.activation(
                                        out=pt.rearrange("p s q -> p (s q)"),
                                        in_=ps.rearrange("p s q -> p (s q)"),
                                        func=F.Exp, scale=SCALE)
                                else:
                                    nc.vector.tensor_scalar(
                                        out=pt.rearrange("p s q -> p (s q)")
                                        .bitcast(i8),
                                        in0=ps.rearrange("p s q -> p (s q)"),
                                        scalar1=ASCH, scalar2=BSCH,
                                        op0=OP.mult, op1=OP.add)
                            else:
                                for rr in range(2):
                                    r = 2 * tp + rr - 4 * ch
                                    c0 = r * PB
                                    nc.vector.scalar_tensor_tensor(
                                        out=pt[:, rr, c0:QC].bitcast(i8),
                                        in0=ps[:, rr, c0:QC],
                                        scalar=BSCH / ASCH,
                                        in1=cza_t[:, r, c0:QC],
                                        op0=OP.add, op1=OP.mult)
                                r0 = 2 * tp - 4 * ch    # 0 or 2
                                nc.gpsimd.memset(
                                    pt[:, 1, r0 * PB:(r0 + 1) * PB], 0)
                            tick()
                    if bg:
                        for f in bg:
                            f()
                        del bg[:]
                    return pts, ptbs

                def attn_pv(ch, pts, ptbs):
                    nfull = 2 * ch if MF else NT // 2
                    ndiag = 2 if MF else 0
                    for b in range(4):
                        jt = 4 * ch + b
                        ctx = pp_ctx.tile([PB, HG, VW], f32, tag="ctx",
                                          name=f"ctx{rep}_{ch}_{b}")
                        for h in range(HG):
                            if MF and ch == 0:
                                for r in range(b + 1):
                                    nc.tensor.matmul(
                                        ctx[:, h, :],
                                        ptbs[(h, r)][:, b * PB:(b + 1) * PB],
                                        v16[r][:, h, :],
                                        start=(r == 0), stop=(r == b),
                                        skip_group_check=True)
                                continue
                            npv = (nfull + 1 + b // 2) if MF else nfull
                            for tp in range(npv):
                                nc.tensor.matmul(
                                    ctx[:, h, :],
                                    pts[(h, tp)][:, :, b * PB:(b + 1) * PB],
                                    v8p[tp][:, :, h, :],
                                    start=(tp == 0),
                                    stop=(tp == npv - 1),
                                    perf_mode=DR, skip_group_check=True)
                        rc4 = rc_pool.tile([PB, HG, 1], f32, tag="rc",
                                           name=f"rc{rep}_{ch}_{b}")
                        nc.vector.reciprocal(rc4, ctx[:, :, HD:HD + 1])
                        if not qm_ones:
                            nc.vector.tensor_scalar_mul(
                                rc4, rc4, qmm_t[:, jt:jt + 1])
                        ot = out_pool.tile([PB, HG, HD], f32, tag="o",
                                           name=f"o{rep}_{ch}_{b}")
                        nc.vector.scalar_tensor_tensor(
                            out=ot, in0=ctx[:, :, 0:HD], scalar=1.0,
                            in1=rc4.broadcast_to([PB, HG, HD]),
                            op0=OP.mult, op1=OP.mult)
                        nc.sync.dma_start(
                            out=out[jt * PB:(jt + 1) * PB, :],
                            in_=ot.rearrange("p g c -> p (g c)"))

                # ---------------- schedule
                def do_proj(chn):
                    xk_t = proj8_qk(chn)
                    proj8_v(chn, xk_t)

                def proj_pieces(chn):
                    # split chunk-chn projection into interleavable pieces
                    state = {}

                    def qk_piece():
                        state["xk"] = proj8_qk(chn)

                    def v_piece():
                        proj8_v(chn, state["xk"])
                    return [qk_piece, v_piece]

                if MF:
                    proj16_qk()
                    sc = {0: attn_scores(0)}
                    proj16_v()
                    proj8_qk(0, do_q=False)
                    do_proj(1)
                    sc[1] = attn_scores(1, bg=proj_pieces(2))
                    attn_pv(0, *sc.pop(0))
                    sc[2] = attn_scores(2, bg=proj_pieces(3))
                    attn_pv(1, *sc.pop(1))
                    sc[3] = attn_scores(3)
                    attn_pv(2, *sc.pop(2))
                    attn_pv(3, *sc.pop(3))
                else:
                    for ch in range(NCH):
                        do_proj(ch)
                    for ch in range(NCH):
                        pts, ptbs = attn_scores(ch)
                        attn_pv(ch, pts, ptbs)

    nc.compile()
    return nc


def _get_nc(mask_future, qk_bias, v_bias, km_ones, qm_ones):
    key = (mask_future, qk_bias, v_bias, km_ones, qm_ones,
           _REPS, _EXPA, _PT8B, _SBUFS, _CTXB)
    if key not in _CACHE:
        _CACHE[key] = _build(mask_future, qk_bias, v_bias, km_ones, qm_ones)
    return _CACHE[key]


def _pack_inputs(query_states, key_states, query_mask, key_mask,
                 Wq, bq, Wk, bk, Wv, bv, mask_future, qk_bias, v_bias,
                 km_ones, qm_ones):
    e4 = ml_dtypes.float8_e4m3
    f2 = np.float16
    in_maps = []
    causal16 = np.zeros((PB, PB), np.float16)
    for p in range(PB):
        causal16[p, :p] = -10000.0
    id16 = np.eye(PB, dtype=np.float16)
    czA = np.zeros((PB, NCH, QC), np.float32)
    for r in range(NCH):
        for p_ in range(PB):
            lo = PB * r + p_
            if lo < QC:
                czA[p_, r, lo:] = ASCH

    for c in range(NCORES):
        b, g = c // BG, c % BG
        s8 = slice(g * HDG, (g + 1) * HDG)
        xq = np.asarray(query_states[b], np.float32)   # [T, D]
        xk = np.asarray(key_states[b], np.float32)
        # [p, ch, dc, t'] = x[ch*512+t', dc*128+p]
        xr_q = xq.T.reshape(KC, PB, NCH, QC).transpose(1, 2, 0, 3)
        xr_k = xk.T.reshape(KC, PB, NCH, QC).transpose(1, 2, 0, 3)
        Wq_g = np.asarray(Wq[s8], np.float32)          # [256, D]
        Wk_g = np.asarray(Wk[s8], np.float32)
        Wv_g = np.asarray(Wv[s8], np.float32)

        def pack_w8(W):
            # [p, dc, s, m=(h*32+dlow)] = W[h*64+32s+dlow, dc*128+p]
            a = W.reshape(HG, 2, 32, KC, PB)           # [h, s, dlow, dc, p]
            return np.ascontiguousarray(
                a.transpose(4, 3, 1, 0, 2).reshape(PB, KC, 2, PB).astype(e4))

        def pack_w16(W):
            # [p, dc, ht, m] = W[ht*128+m, dc*128+p]
            a = W.reshape(2, PB, KC, PB)               # [ht, m, dc, p]
            return np.ascontiguousarray(a.transpose(3, 2, 0, 1).astype(f2))

        def pack_wv(W, dt):
            a = W.reshape(HDG, KC, PB)                 # [m, dc, p]
            return np.ascontiguousarray(a.transpose(2, 1, 0).astype(dt))

        m = {
            "xq8": np.ascontiguousarray(
                (xr_q[:, 1:] if mask_future else xr_q).astype(e4)),
            "xk8": np.ascontiguousarray(xr_k.astype(e4)),
            "wq8": pack_w8(Wq_g),
            "wk8": pack_w8(Wk_g),
            "wv8": pack_wv(Wv_g, e4),
        }
        if mask_future:
            m["xq16"] = np.ascontiguousarray(xr_q[:, 0].astype(f2))
            m["xk16"] = np.ascontiguousarray(xr_k[:, 0].astype(f2))
            m["wq16"] = pack_w16(Wq_g)
            m["wk16"] = pack_w16(Wk_g)
            m["wv16"] = pack_wv(Wv_g, f2)
            m["c16"] = causal16
            m["id16"] = id16
            m["cza"] = czA
        if not km_ones:
            km = np.asarray(key_mask[b], np.float32)
            m["ekm"] = np.ascontiguousarray(
                np.exp((km - 1.0) * 10000.0).reshape(NT, PB).T
                .astype(np.float32))
        if not qm_ones:
            qm = np.asarray(query_mask[b], np.float32)
            m["qmm"] = np.ascontiguousarray(qm.reshape(NT, PB).T)
        if qk_bias or v_bias:
            m["ones_r"] = np.ones((1, QC), f2)
        if qk_bias:
            bq_g = np.asarray(bq[s8], np.float32)
            bk_g = np.asarray(bk[s8], np.float32)

            def pack_b8(bb):
                a = bb.reshape(HG, 2, 32)              # [h, s, dlow]
                return np.ascontiguousarray(
                    a.transpose(1, 0, 2).reshape(1, 2, PB).astype(f2))

            m["bq8"] = pack_b8(bq_g)
            m["bk8"] = pack_b8(bk_g)
            if mask_future:
                m["bq16"] = np.ascontiguousarray(
                    bq_g.reshape(1, 2, PB).astype(f2))
                m["bk16"] = np.ascontiguousarray(
                    bk_g.reshape(1, 2, PB).astype(f2))
        if v_bias:
            m["bvr"] = np.ascontiguousarray(
                np.asarray(bv[s8], np.float32).reshape(1, HDG).astype(f2))
        in_maps.append(m)
    return in_maps


def kernel(query_states, key_states, query_mask, key_mask,
           Wq, bq, Wk, bk, Wv, bv, mask_future):
    query_states = np.asarray(query_states, np.float32)
    key_states = np.asarray(key_states, np.float32)
    mask_future = bool(int(np.asarray(mask_future)))
    qk_bias = bool(np.any(np.asarray(bq)) or np.any(np.asarray(bk)))
    v_bias = bool(np.any(np.asarray(bv)))
    km_ones = bool(np.all(np.asarray(key_mask) == 1.0))
    qm_ones = bool(np.all(np.asarray(query_mask) == 1.0))

    nc = _get_nc(mask_future, qk_bias, v_bias, km_ones, qm_ones)
    in_maps = _pack_inputs(query_states, key_states, query_mask, key_mask,
                           Wq, bq, Wk, bk, Wv, bv, mask_future, qk_bias,
                           v_bias, km_ones, qm_ones)
    res = run_bass_kernel_spmd(nc, in_maps, core_ids=list(range(NCORES)))
    full = np.empty((B, T, D), np.float32)
    for c in range(NCORES):
        b, g = c // BG, c % BG
        full[b][:, g * HDG:(g + 1) * HDG] = res.results[c]["out"]
    return full


def modeled_time_ns():
    """Cost-model (TimelineSim) estimate for the current cached module."""
    from concourse.timeline_sim import TimelineSim
    nc = next(iter(_CACHE.values()))
    return TimelineSim(nc, no_exec=True).simulate()


# revision 15
# speedup vs baseline: 1.5946x; 1.2207x over previous
"""Sharded attention kernel for Trainium2 (8 NeuronCores), v5.

Problem: B=2, T=2048, D=1024, H=16 heads (head dim 64), causal self-attention
with separate Q/K/V projections, key-mask additive bias and post-softmax
query-mask, fp32 reference; tolerance max-rel 2e-2 vs global max.

Sharding: data-parallel over the 2 batches x tensor-parallel over 4 head
groups (4 heads each) -> 8 fully independent cores, no collectives.

Per-core plan (all big matmuls fp8e4m3 DoubleRow = 0.5 cycles/row with 256-
deep contraction; accuracy recovered by an fp16 path for rows < 512 whose
softmax support is small):
  - Q/K projections emit q^T/k^T directly in the DoubleRow d-split layout
    [128p = 4 heads x 32 d_low, 2 slots, t] via host-packed weight column
    order; V projection emits natural [t_k, dims] strips. fp16 plain-layout
    projections cover chunk-0 q/k and v strips 0-3 (when causal).
  - scores S^T[k, q] per 128-row k-strip: fp8-DR (256 cyc per 512-wide
    strip), fp16 non-DR for chunk-0 diagonal. The causal triangle of a
    diagonal block is handled by a -10000 bias matmul into PSUM (ACT-exp
    path) or a {A,0} mask operand inside the DVE Schraudolph.
  - softmax: no max-subtraction (scores bounded); exp work is split between
    the ACT engine (true exp, fp8 output) and the DVE (Schraudolph int8
    bit-trick: round(1.4427*s + 55.65) written as int8 == fp8e4m3 bits of
    ~exp(s/8); hw rounds-to-nearest and saturates; masked cells -> 0).
    Key-mask bias exp(b_k) is folded multiplicatively into the V rows
    (including the denominator ones-column), so exp needs no bias operand
    and strip-pairs fuse into single [128, 1024] instructions.
  - PV runs in natural layout: probs (stationary, [128k, 2, 128t] slices of
    pair tiles) x V (moving, [128k, 2, 68]) -> ctx[t, dims] accumulated over
    strips; 34 cycles per pair per 128-row t-block. The ones-column gives
    the denominator in ctx column 64; normalization is a per-partition
    reciprocal + one stride-0-broadcast multiply per (chunk, t-block).
"""

import os
import sys

import numpy as np

for _p in ("/opt/trn_rl_repo",):
    if os.path.isdir(_p) and _p not in sys.path:
        sys.path.append(_p)

import ml_dtypes  # noqa: E402

import concourse.mybir as mybir  # noqa: E402
import concourse.tile as tile  # noqa: E402
from concourse import bacc  # noqa: E402
from concourse.bass_utils import run_bass_kernel_spmd  # noqa: E402

B, T, D, H = 2, 2048, 1024, 16
HD = D // H          # 64 head dim
NCORES = 8
BG = NCORES // B     # 4 head-groups per batch
HG = H // BG         # 4 heads per core
HDG = HG * HD        # 256 projection cols per core
PB = 128             # partition block
NT = T // PB         # 16 k-strips
QC = 512             # q-chunk width
NCH = T // QC        # 4 q-chunks
KC = D // PB         # 8 contraction chunks
VW = 68              # padded v width: 64 dims + ones col + 3 pad (4B align)
SCALE = 0.125

# Schraudolph fp8e4m3 bit-trick constants (hw float->int8 rounds to nearest)
ASCH = 8.0 * 1.4426950408889634 * SCALE     # = 1.4427 on raw scores
BSCH = 56.0 - 0.35                           # exponent-bias offset, tuned

f32 = mybir.dt.float32
fp8 = mybir.dt.float8e4
fp16 = mybir.dt.float16
bf16 = mybir.dt.bfloat16
i8 = mybir.dt.int8
F = mybir.ActivationFunctionType
DR = mybir.MatmulPerfMode.DoubleRow
OP = mybir.AluOpType

_CACHE: dict = {}
_REPS = int(os.environ.get("K_REPS", "1"))
# percentage of off-diagonal exp pairs handled by ACT (rest on DVE)
_EXPA = int(os.environ.get("K_EXPA", "68"))
_PT8B = int(os.environ.get("K_PT8B", "60"))
_PTBB = int(os.environ.get("K_PTBB", "17"))
_SBUFS = int(os.environ.get("K_SB", "3"))
_CTXB = int(os.environ.get("K_CTX", "1"))
_OUTB = int(os.environ.get("K_OUTB", "4"))
_PRJB = int(os.environ.get("K_PRJ", "1"))
_X8B = int(os.environ.get("K_X8B", "4"))


def _build(mask_future: bool, qk_bias: bool, v_bias: bool, km_ones: bool,
           qm_ones: bool):
    nc = bacc.Bacc("TRN2", target_bir_lowering=False, debug=False,
                   num_devices=NCORES)
    MF = mask_future
    NQ8 = NCH - 1 if MF else NCH       # q8 chunks (chunk 0 via fp16 if MF)

    def din(name, shape, dt):
        return nc.dram_tensor(name, shape, dt, kind="ExternalInput").ap()

    xq8 = din("xq8", [PB, NQ8, KC, QC], fp8)
    xk8 = din("xk8", [PB, NCH, KC, QC], fp8)
    wq8 = din("wq8", [PB, KC, 2, PB], fp8)
    wk8 = din("wk8", [PB, KC, 2, PB], fp8)
    wv8 = din("wv8", [PB, KC, HDG], fp8)
    xq16 = xk16 = wq16 = wk16 = wv16 = c16 = id16 = None
    if MF:
        xq16 = din("xq16", [PB, KC, QC], fp16)         # chunk 0
        xk16 = din("xk16", [PB, KC, QC], fp16)
        wq16 = din("wq16", [PB, KC, 2, PB], fp16)
        wk16 = din("wk16", [PB, KC, 2, PB], fp16)
        wv16 = din("wv16", [PB, KC, HDG], fp16)
        c16 = din("c16", [PB, PB], fp16)               # upper-tri -10000
        id16 = din("id16", [PB, PB], fp16)
    ekm = qmm = None
    if not km_ones:
        ekm = din("ekm", [PB, NT], f32)
    if not qm_ones:
        qmm = din("qmm", [PB, NT], f32)
    bq8 = bk8 = bq16 = bk16 = bvr = ones_r = None
    if qk_bias or v_bias:
        ones_r = din("ones_r", [1, QC], fp16)
    if qk_bias:
        bq8 = din("bq8", [1, 2, PB], fp16)   # d-split order biases
        bk8 = din("bk8", [1, 2, PB], fp16)
        if MF:
            bq16 = din("bq16", [1, 2, PB], fp16)  # plain order
            bk16 = din("bk16", [1, 2, PB], fp16)
    if v_bias:
        bvr = din("bvr", [1, HDG], fp16)

    out = nc.dram_tensor("out", [T, HDG], f32, kind="ExternalOutput").ap()

    with tile.TileContext(nc) as tc:
        with (
            tc.tile_pool(name="singles", bufs=1) as S,
            tc.tile_pool(name="x8", bufs=_X8B) as x8_pool,
            tc.tile_pool(name="qT8", bufs=(2 if MF else NCH)) as qT8_pool,
            tc.tile_pool(name="pt8", bufs=_PT8B) as pt8_pool,
            tc.tile_pool(name="ptb", bufs=_PTBB) as ptb_pool,
            tc.tile_pool(name="outs", bufs=_OUTB) as out_pool,
            tc.tile_pool(name="rc", bufs=4) as rc_pool,
            tc.tile_pool(name="pp_prj", bufs=_PRJB, space="PSUM") as pp_prj,
            tc.tile_pool(name="pp_s", bufs=_SBUFS, space="PSUM") as pp_s,
            tc.tile_pool(name="pp_ctx", bufs=_CTXB, space="PSUM") as pp_ctx,
        ):
            # ---------------- constants / weights / persistent tiles
            # fp16 chunk-0 inputs first: they head the critical path
            w = {}
            xq16_t = xk16_t = c16_t = id16_t = None
            if MF:
                w["q16"] = S.tile([PB, KC, 2, PB], fp16, tag="wq16", name="wq16t")
                nc.sync.dma_start(out=w["q16"], in_=wq16)
                xq16_t = S.tile([PB, KC, QC], fp16, tag="xq16")
                nc.sync.dma_start(out=xq16_t[:, 0:KC // 2, :],
                                  in_=xq16[:, 0:KC // 2, :])
                nc.sync.dma_start(out=xq16_t[:, KC // 2:, :],
                                  in_=xq16[:, KC // 2:, :])
                w["k16"] = S.tile([PB, KC, 2, PB], fp16, tag="wk16", name="wk16t")
                nc.sync.dma_start(out=w["k16"], in_=wk16)
                xk16_t = S.tile([PB, KC, QC], fp16, tag="xk16")
                nc.sync.dma_start(out=xk16_t[:, 0:KC // 2, :],
                                  in_=xk16[:, 0:KC // 2, :])
                nc.sync.dma_start(out=xk16_t[:, KC // 2:, :],
                                  in_=xk16[:, KC // 2:, :])
                c16_t = S.tile([PB, PB], fp16, tag="c16")
                nc.sync.dma_start(out=c16_t, in_=c16)
                id16_t = S.tile([PB, PB], fp16, tag="id16")
                nc.sync.dma_start(out=id16_t, in_=id16)

            for name, src, shp, dt in (
                    ("q8", wq8, [PB, KC, 2, PB], fp8),
                    ("k8", wk8, [PB, KC, 2, PB], fp8),
                    ("v8", wv8, [PB, KC, HDG], fp8)):
                t = S.tile(shp, dt, tag=f"w{name}")
                nc.sync.dma_start(out=t, in_=src)
                w[name] = t
            if MF:
                w["v16"] = S.tile([PB, KC, HDG], fp16, tag="wv16",
                                  name="wv16t")
                nc.sync.dma_start(out=w["v16"], in_=wv16)
            ekm_t = qmm_t = None
            if not km_ones:
                ekm_t = S.tile([PB, NT], f32, tag="ekm")
                nc.sync.dma_start(out=ekm_t, in_=ekm)
            if not qm_ones:
                qmm_t = S.tile([PB, NT], f32, tag="qmm")
                nc.sync.dma_start(out=qmm_t, in_=qmm)
            ones_t = None
            if ones_r is not None:
                ones_t = S.tile([1, QC], fp16, tag="ones")
                nc.sync.dma_start(out=ones_t, in_=ones_r)
            b_t = {}
            for nm, src in (("bq8", bq8), ("bk8", bk8), ("bq16", bq16),
                            ("bk16", bk16)):
                if src is not None:
                    b_t[nm] = S.tile([1, 2, PB], fp16, tag=nm)
                    nc.sync.dma_start(out=b_t[nm], in_=src)
            if bvr is not None:
                b_t["bvr"] = S.tile([1, HDG], fp16, tag="bvr")
                nc.sync.dma_start(out=b_t["bvr"], in_=bvr)

            _NWARM = int(os.environ.get("K_WARM", "0"))
            if _NWARM:
                warm = S.tile([PB, QC], bf16, tag="warm")
                nc.gpsimd.memset(warm, 0)
                wps = pp_prj.tile([PB, QC], f32, tag="prj", name="warmps")
                for _wi in range(_NWARM):
                    nc.tensor.matmul(wps, warm[:, 0:PB], warm,
                                     start=(_wi == 0), stop=False,
                                     skip_group_check=True)
                nc.tensor.matmul(wps, warm[:, 0:PB], warm,
                                 start=False, stop=True,
                                 skip_group_check=True)

            for rep in range(_REPS):
                kT8 = [S.tile([PB, 2, QC], fp8, tag=f"kT8_{c}",
                              name=f"kT8_{rep}_{c}") for c in range(NCH)]
                kT16 = qT16 = v16 = None
                if MF:
                    kT16 = [S.tile([PB, QC], fp16, tag=f"kT16_{i}",
                                   name=f"kT16_{rep}_{i}") for i in range(2)]
                    qT16 = [S.tile([PB, QC], fp16, tag=f"qT16_{i}",
                                   name=f"qT16_{rep}_{i}") for i in range(2)]
                    v16 = [S.tile([PB, HG, VW], bf16, tag=f"v16_{r}",
                                  name=f"v16_{rep}_{r}") for r in range(4)]
                v8p = [S.tile([PB, 2, HG, VW], fp8, tag=f"v8p_{t}",
                              name=f"v8p_{rep}_{t}") for t in range(NT // 2)]
                qT8 = {}  # ch -> tile

                def scalar_km(i):
                    if km_ones:
                        return None
                    return ekm_t[:, i:i + 1]

                # ---------------- projections
                def _store_v8(i, ps):
                    t, s = i // 2, i % 2
                    km = scalar_km(i)
                    ps3 = ps[:, 0:HDG].rearrange("p (g c) -> p g c", c=HD)
                    if km is None:
                        nc.vector.tensor_copy(v8p[t][:, s, :, 0:HD], ps3)
                        nc.gpsimd.memset(v8p[t][:, s, :, HD:HD + 1], 1.0)
                    else:
                        nc.vector.tensor_scalar_mul(
                            v8p[t][:, s, :, 0:HD], ps3, km)
                        nc.vector.tensor_copy(
                            v8p[t][:, s, :, HD:HD + 1],
                            km.rearrange("p () -> p () ()")
                            .broadcast_to([PB, HG, 1]))

                def proj16_qk():
                    for ht, (wname, x_t, dst, bias) in (
                            (0, ("q16", xq16_t, qT16, "bq16")),
                            (0, ("k16", xk16_t, kT16, "bk16")),
                            (1, ("q16", xq16_t, qT16, "bq16")),
                            (1, ("k16", xk16_t, kT16, "bk16"))):
                        if True:
                            ps = pp_prj.tile([PB, QC], f32, tag="prj",
                                             name=f"p16{rep}_{wname}_{ht}")
                            for dc in range(KC):
                                nc.tensor.matmul(
                                    ps, w[wname][:, dc, ht, :],
                                    x_t[:, dc, :],
                                    start=(dc == 0),
                                    stop=(dc == KC - 1 and not qk_bias))
                            if qk_bias:
                                nc.tensor.matmul(
                                    ps, b_t[bias][:, ht, :], ones_t,
                                    start=False, stop=True)
                            nc.vector.tensor_copy(dst[ht], ps)

                def proj16_v():
                    for r in range(4):
                        ps = pp_prj.tile([PB, QC], f32, tag="prj",
                                         name=f"pv16{rep}_{r}")
                        for dc in range(KC):
                            nc.tensor.matmul(
                                ps[:, 0:HDG],
                                xk16_t[:, dc, r * PB:(r + 1) * PB],
                                w["v16"][:, dc, :],
                                start=(dc == 0),
                                stop=(dc == KC - 1 and not v_bias))
                        if v_bias:
                            nc.tensor.matmul(
                                ps[:, 0:HDG], ones_t[:, 0:PB], b_t["bvr"],
                                start=False, stop=True)
                        km = scalar_km(r)
                        ps3 = ps[:, 0:HDG].rearrange("p (g c) -> p g c", c=HD)
                        if km is None:
                            nc.vector.tensor_copy(v16[r][:, :, 0:HD], ps3)
                            nc.gpsimd.memset(v16[r][:, :, HD:HD + 1], 1.0)
                        else:
                            nc.vector.tensor_scalar_mul(
                                v16[r][:, :, 0:HD], ps3, km)
                            nc.vector.tensor_copy(
                                v16[r][:, :, HD:HD + 1],
                                km.rearrange("p () -> p () ()")
                                .broadcast_to([PB, HG, 1]))
                        _store_v8(r, ps)

                def proj8_qk(ch, do_q=True):
                    """DR projections for chunk ch; returns k x-tile for the
                    v projection. q skipped for chunk 0 when causal."""
                    jobs = []
                    if do_q:
                        qt = qT8_pool.tile([PB, 2, QC], fp8, tag="qT8",
                                           name=f"qT8_{rep}_{ch}")
                        qT8[ch] = qt
                        jobs.append(("q8", xq8[:, ch - 1 if MF else ch],
                                     qt, "bq8"))
                    jobs.append(("k8", xk8[:, ch], kT8[ch], "bk8"))
                    xk_t = None
                    for wname, src, dst, bias in jobs:
                        x_t = x8_pool.tile([PB, KC, QC], fp8, tag="x8",
                                           name=f"x8_{rep}_{ch}_{wname}")
                        nc.sync.dma_start(out=x_t, in_=src)
                        if wname == "k8":
                            xk_t = x_t
                        for s in range(2):
                            ps = pp_prj.tile([PB, QC], f32, tag="prj",
                                             name=f"p8{rep}_{ch}_{wname}_{s}")
                            for d2 in range(KC // 2):
                                nc.tensor.matmul(
                                    ps,
                                    w[wname][:, 2 * d2:2 * d2 + 2, s, :],
                                    x_t[:, 2 * d2:2 * d2 + 2, :],
                                    start=(d2 == 0),
                                    stop=(d2 == KC // 2 - 1 and not qk_bias),
                                    perf_mode=DR)
                            if qk_bias:
                                nc.tensor.matmul(
                                    ps, b_t[bias][:, s, :], ones_t,
                                    start=False, stop=True)
                            nc.vector.tensor_copy(dst[:, s, :], ps)
                    return xk_t

                def proj8_v(ch, xk_t):
                    for r in range(4):
                        i = 4 * ch + r
                        if MF and ch == 0:
                            continue  # strips 0-3 via fp16 path
                        ps = pp_prj.tile([PB, QC], f32, tag="prj",
                                         name=f"pv{rep}_{i}")
                        for d2 in range(KC // 2):
                            nc.tensor.matmul(
                                ps[:, 0:HDG],
                                xk_t[:, 2 * d2:2 * d2 + 2,
                                     r * PB:(r + 1) * PB],
                                w["v8"][:, 2 * d2:2 * d2 + 2, :],
                                start=(d2 == 0),
                                stop=(d2 == KC // 2 - 1 and not v_bias),
                                perf_mode=DR)
                        if v_bias:
                            nc.tensor.matmul(
                                ps[:, 0:HDG], ones_t[:, 0:PB], b_t["bvr"],
                                start=False, stop=True)
                        _store_v8(i, ps)

                # ---------------- attention
                _acc = [0.0]

                def pick_engine():
                    _acc[0] += _EXPA / 100.0
                    if _acc[0] >= 1.0:
                        _acc[0] -= 1.0
                        return "act"
                    return "dve"

                def attn_scores(ch, bg=None):
                    nfull = 2 * ch if MF else NT // 2
                    ndiag = 2 if MF else 0
                    pts = {}
                    ptbs = {}
                    nemit = [0]

                    def tick():
                        nemit[0] += 1
                        if bg and nemit[0] % 2 == 0 and bg:
                            bg.pop(0)()
                    for h in range(HG):
                        if MF and ch == 0:
                            # fp16 scores + ACT exp -> bf16, per strip
                            for tp in range(2):
                                ps = pp_s.tile(
                                    [PB, 2, QC], f32, tag="s",
                                    name=f"s{rep}_0_{h}_{tp}")
                                for rr in range(2):
                                    r = 2 * tp + rr
                                    c0 = r * PB
                                    h2, hb = h // 2, (h % 2) * HD
                                    nc.tensor.matmul(
                                        ps[:, rr, c0:QC],
                                        kT16[h2][hb:hb + HD,
                                                 r * PB:(r + 1) * PB],
                                        qT16[h2][hb:hb + HD, c0:QC],
                                        start=True, stop=False)
                                    nc.tensor.matmul(
                                        ps[:, rr, c0:c0 + PB],
                                        id16_t, c16_t,
                                        start=False, stop=True)
                                    pb = ptb_pool.tile(
                                        [PB, QC], bf16, tag="pb",
                                        name=f"pb{rep}_{h}_{r}")
                                    nc.scalar.activation(
                                        out=pb[:, c0:QC],
                                        in_=ps[:, rr, c0:QC],
                                        func=F.Exp, scale=SCALE)
                                    ptbs[(h, r)] = pb
                                tick()
                            continue
                        for tp in range(nfull + ndiag):
                            diag = MF and tp >= nfull
                            ps = pp_s.tile([PB, 2, QC], f32, tag="s",
                                           name=f"s{rep}_{ch}_{h}_{tp}")
                            pt = pt8_pool.tile([PB, 2, QC], fp8, tag="pt",
                                               name=f"pt{rep}_{ch}_{h}_{tp}")
                            pts[(h, tp)] = pt
                            for rr in range(2):
                                i = 2 * tp + rr
                                c0 = (i - 4 * ch) * PB if diag else 0
                                ck, rb = i // 4, i % 4
                                nc.tensor.matmul(
                                    ps[:, rr, c0:QC],
                                    kT8[ck][32 * h:32 * h + 32, :,
                                            rb * PB:(rb + 1) * PB],
                                    qT8[ch][32 * h:32 * h + 32, :, c0:QC],
                                    start=True, stop=not diag,
                                    perf_mode=DR, tile_position=(32 * h, 0))
                                if diag:
                                    # -10000 on the causal triangle; exp and
                                    # int8 saturation then zero those cells
                                    nc.tensor.matmul(
                                        ps[:, rr, c0:c0 + PB], id16_t, c16_t,
                                        start=False, stop=True)
                            cA = (2 * tp - 4 * ch) * PB if diag else 0
                            pss = ps[:, :, cA:]
                            pts_ = pt[:, :, cA:]
                            if pick_engine() == "act":
                                nc.scalar.activation(
                                    out=pts_, in_=pss,
                                    func=F.Exp, scale=SCALE)
                            else:
                                nc.vector.tensor_scalar(
                                    out=pts_.bitcast(i8), in0=pss,
                                    scalar1=ASCH, scalar2=BSCH,
                                    op0=OP.mult, op1=OP.add)
                            if diag:
                                r0 = 2 * tp - 4 * ch    # 0 or 2
                                nc.gpsimd.memset(
                                    pt[:, 1, r0 * PB:(r0 + 1) * PB], 0)
                            tick()
                    if bg:
                        for f in bg:
                            f()
                        del bg[:]
                    return pts, ptbs

                def attn_pv(ch, pts, ptbs):
                    nfull = 2 * ch if MF else NT // 2
                    ndiag = 2 if MF else 0
                    for b in range(4):
                        jt = 4 * ch + b
                        ctx = pp_ctx.tile([PB, HG, VW], f32, tag="ctx",
                                          name=f"ctx{rep}_{ch}_{b}")
                        for h in range(HG):
                            if MF and ch == 0:
                                for r in range(b + 1):
                                    nc.tensor.matmul(
                                        ctx[:, h, :],
                                        ptbs[(h, r)][:, b * PB:(b + 1) * PB],
                                        v16[r][:, h, :],
                                        start=(r == 0), stop=(r == b),
                                        skip_group_check=True)
                                continue
                            npv = (nfull + 1 + b // 2) if MF else nfull
                            for tp in range(npv):
                                nc.tensor.matmul(
                                    ctx[:, h, :],
                                    pts[(h, tp)][:, :, b * PB:(b + 1) * PB],
                                    v8p[tp][:, :, h, :],
                                    start=(tp == 0),
                                    stop=(tp == npv - 1),
                                    perf_mode=DR, skip_group_check=True)
                        rc4 = rc_pool.tile([PB, HG, 1], f32, tag="rc",
                                           name=f"rc{rep}_{ch}_{b}")
                        nc.vector.reciprocal(rc4, ctx[:, :, HD:HD + 1])
                        if not qm_ones:
                            nc.vector.tensor_scalar_mul(
                                rc4, rc4, qmm_t[:, jt:jt + 1])
                        ot = out_pool.tile([PB, HG, HD], f32, tag="o",
                                           name=f"o{rep}_{ch}_{b}")
                        nc.vector.scalar_tensor_tensor(
                            out=ot, in0=ctx[:, :, 0:HD], scalar=1.0,
                            in1=rc4.broadcast_to([PB, HG, HD]),
                            op0=OP.mult, op1=OP.mult)
                        nc.sync.dma_start(
                            out=out[jt * PB:(jt + 1) * PB, :],
                            in_=ot.rearrange("p g c -> p (g c)"))

                # ---------------- schedule
                def do_proj(chn):
                    xk_t = proj8_qk(chn)
                    proj8_v(chn, xk_t)

                def v_piece(chn, xk_t):
                    def emit():
                        proj8_v(chn, xk_t)
                    return [emit]

                if MF:
                    proj16_qk()
                    sc = {0: attn_scores(0)}
                    proj8_qk(0, do_q=False)
                    do_proj(1)
                    xk2 = proj8_qk(2)
                    sc[1] = attn_scores(1, bg=v_piece(2, xk2))
                    proj16_v()
                    attn_pv(0, *sc.pop(0))
                    xk3 = proj8_qk(3)
                    sc[2] = attn_scores(2, bg=v_piece(3, xk3))
                    attn_pv(1, *sc.pop(1))
                    sc[3] = attn_scores(3)
                    attn_pv(2, *sc.pop(2))
                    attn_pv(3, *sc.pop(3))
                else:
                    for ch in range(NCH):
                        do_proj(ch)
                    for ch in range(NCH):
                        pts, ptbs = attn_scores(ch)
                        attn_pv(ch, pts, ptbs)

    nc.compile()
    return nc


def _get_nc(mask_future, qk_bias, v_bias, km_ones, qm_ones):
    key = (mask_future, qk_bias, v_bias, km_ones, qm_ones,
           _REPS, _EXPA, _PT8B, _SBUFS, _CTXB)
    if key not in _CACHE:
        _CACHE[key] = _build(mask_future, qk_bias, v_bias, km_ones, qm_ones)
    return _CACHE[key]


def _pack_inputs(query_states, key_states, query_mask, key_mask,
                 Wq, bq, Wk, bk, Wv, bv, mask_future, qk_bias, v_bias,
                 km_ones, qm_ones):
    e4 = ml_dtypes.float8_e4m3
    f2 = np.float16
    in_maps = []
    causal16 = np.zeros((PB, PB), np.float16)
    for p in range(PB):
        causal16[p, :p] = -10000.0
    id16 = np.eye(PB, dtype=np.float16)
    for c in range(NCORES):
        b, g = c // BG, c % BG
        s8 = slice(g * HDG, (g + 1) * HDG)
        xq = np.asarray(query_states[b], np.float32)   # [T, D]
        xk = np.asarray(key_states[b], np.float32)
        # [p, ch, dc, t'] = x[ch*512+t', dc*128+p]
        xr_q = xq.T.reshape(KC, PB, NCH, QC).transpose(1, 2, 0, 3)
        xr_k = xk.T.reshape(KC, PB, NCH, QC).transpose(1, 2, 0, 3)
        Wq_g = np.asarray(Wq[s8], np.float32)          # [256, D]
        Wk_g = np.asarray(Wk[s8], np.float32)
        Wv_g = np.asarray(Wv[s8], np.float32)

        def pack_w8(W):
            # [p, dc, s, m=(h*32+dlow)] = W[h*64+32s+dlow, dc*128+p]
            a = W.reshape(HG, 2, 32, KC, PB)           # [h, s, dlow, dc, p]
            return np.ascontiguousarray(
                a.transpose(4, 3, 1, 0, 2).reshape(PB, KC, 2, PB).astype(e4))

        def pack_w16(W):
            # [p, dc, ht, m] = W[ht*128+m, dc*128+p]
            a = W.reshape(2, PB, KC, PB)               # [ht, m, dc, p]
            return np.ascontiguousarray(a.transpose(3, 2, 0, 1).astype(f2))

        def pack_wv(W, dt):
            a = W.reshape(HDG, KC, PB)                 # [m, dc, p]
            return np.ascontiguousarray(a.transpose(2, 1, 0).astype(dt))

        m = {
            "xq8": np.ascontiguousarray(
                (xr_q[:, 1:] if mask_future else xr_q).astype(e4)),
            "xk8": np.ascontiguousarray(xr_k.astype(e4)),
            "wq8": pack_w8(Wq_g),
            "wk8": pack_w8(Wk_g),
            "wv8": pack_wv(Wv_g, e4),
        }
        if mask_future:
            m["xq16"] = np.ascontiguousarray(xr_q[:, 0].astype(f2))
            m["xk16"] = np.ascontiguousarray(xr_k[:, 0].astype(f2))
            m["wq16"] = pack_w16(Wq_g)
            m["wk16"] = pack_w16(Wk_g)
            m["wv16"] = pack_wv(Wv_g, f2)
            m["c16"] = causal16
            m["id16"] = id16
        if not km_ones:
            km = np.asarray(key_mask[b], np.float32)
            m["ekm"] = np.ascontiguousarray(
                np.exp((km - 1.0) * 10000.0).reshape(NT, PB).T
                .astype(np.float32))
        if not qm_ones:
            qm = np.asarray(query_mask[b], np.float32)
            m["qmm"] = np.ascontiguousarray(qm.reshape(NT, PB).T)
        if qk_bias or v_bias:
            m["ones_r"] = np.ones((1, QC), f2)
        if qk_bias:
            bq_g = np.asarray(bq[s8], np.float32)
            bk_g = np.asarray(bk[s8], np.float32)

            def pack_b8(bb):
                a = bb.reshape(HG, 2, 32)              # [h, s, dlow]
                return np.ascontiguousarray(
                    a.transpose(1, 0, 2).reshape(1, 2, PB).astype(f2))

            m["bq8"] = pack_b8(bq_g)
            m["bk8"] = pack_b8(bk_g)
            if mask_future:
                m["bq16"] = np.ascontiguousarray(
                    bq_g.reshape(1, 2, PB).astype(f2))
                m["bk16"] = np.ascontiguousarray(
                    bk_g.reshape(1, 2, PB).astype(f2))
        if v_bias:
            m["bvr"] = np.ascontiguousarray(
                np.asarray(bv[s8], np.float32).reshape(1, HDG).astype(f2))
        in_maps.append(m)
    return in_maps


def kernel(query_states, key_states, query_mask, key_mask,
           Wq, bq, Wk, bk, Wv, bv, mask_future):
    query_states = np.asarray(query_states, np.float32)
    key_states = np.asarray(key_states, np.float32)
    mask_future = bool(int(np.asarray(mask_future)))
    qk_bias = bool(np.any(np.asarray(bq)) or np.any(np.asarray(bk)))
    v_bias = bool(np.any(np.asarray(bv)))
    km_ones = bool(np.all(np.asarray(key_mask) == 1.0))
    qm_ones = bool(np.all(np.asarray(query_mask) == 1.0))

    nc = _get_nc(mask_future, qk_bias, v_bias, km_ones, qm_ones)
    in_maps = _pack_inputs(query_states, key_states, query_mask, key_mask,
                           Wq, bq, Wk, bk, Wv, bv, mask_future, qk_bias,
                           v_bias, km_ones, qm_ones)
    res = run_bass_kernel_spmd(nc, in_maps, core_ids=list(range(NCORES)))
    full = np.empty((B, T, D), np.float32)
    for c in range(NCORES):
        b, g = c // BG, c % BG
        full[b][:, g * HDG:(g + 1) * HDG] = res.results[c]["out"]
    return full


def modeled_time_ns():
    """Cost-model (TimelineSim) estimate for the current cached module."""
    from concourse.timeline_sim import TimelineSim
    nc = next(iter(_CACHE.values()))
    return TimelineSim(nc, no_exec=True).simulate()
